# revision 22
# baseline (speedup 1.0000x reference)
"""Trainium2 Bass kernel for nn_AttentionResidualBlock.

Computation (per token t, head h):
    q = x @ W_q + b_q
    scores[t,h,l] = <q[t,h,:], k[t,l,h,:]> / sqrt(hd)   (k = layer_history)
    w = softmax_l(scores)
    out[t,h,:] = sum_l w[t,h,l] * k[t,l,h,:]

Sharding: data-parallel over the 8192 (b,s) tokens -> 8 cores x 1024 tokens.
Per-core layout: token-major (tokens on partitions), 8 tiles of 128 tokens,
each tile processed as two independent head-halves (h 0:8 / 8:16) so the
serial softmax middle has fine grain and pipelines deeply across units.

v3 vs the 227us baseline (timeline-sim predicts 223us baseline -> 200us):
  - all inputs fp16 on host instead of fp32+cast-DMA: HBM traffic halves
    (DMA ~21us/tile -> ~10.5us/tile) and rel err improves 10x to 1.2e-3
    (fp16 mantissa 10 bits vs bf16 7)
  - all DMA on HWDGE queues (sync/scalar); GPSIMD freed for compute
  - GPSIMD runs the weighted-sum mul for 6 of 12 layers via tensor_tensor
    (never contends with DVE TT for the shared SBUF port); DVE keeps the
    scores mul + fold tree + softmax small ops + the other 6 layers
  - each 128-token tile is two independent head-half units (softmax
    couples l, not h) for fine-grained cross-unit pipelining
  - W / k are split into per-chunk tiles (dependency tracking is per-tile,
    so consumers only wait for their own chunk's DMA); the prologue DMAs
    are ordered bq/ones/xt0/W-h0/k0a/W-h1/... on one queue so the first
    q_proj and scores start ~8us in
  - q PSUM->SBUF scale copy on ACT; prod bufs=3 for ~2 tiles of skew;
    the last tile runs all-DVE so the tail isn't gated on gpsimd
Engine busy (sim): DVE 151us (bottleneck), Pool ~105, PE ~96, ACT ~81,
DMA ~99; total 200us = busy + 13us prologue + DMA ramp + 6us tail.
"""

import math
from contextlib import ExitStack

import numpy as np

import concourse.tile as tile
from concourse import bacc, mybir
from concourse.bass_utils import run_bass_kernel_spmd
from concourse import masks

FP32 = mybir.dt.float32
FP16 = mybir.dt.float16

B, S, L, D, H = 4, 2048, 12, 1024, 16
HD = D // H
N_CORES = 8
T = B * S // N_CORES          # tokens per core = 1024
P = 128                       # partition tile
NT = T // P                   # 8 token tiles per core
SCALE = 1.0 / math.sqrt(HD)   # 0.125
HH = H // 2                   # heads per half = 8
DH = D // 2                   # feature cols per half = 512

# engine split tuning: layers handled by GPSIMD instead of DVE
GF = 0    # fold-tree layers on gpsimd (of L=12)
GP2 = 8   # weighted-sum mul layers on gpsimd (of L=12)


def build_body(ctx, tc, out, xt, kh, wq, bq, ones, repeat=1):
    nc = tc.nc

    const_pool = ctx.enter_context(tc.tile_pool(name="const", bufs=1))
    xtp = ctx.enter_context(tc.tile_pool(name="xt", bufs=2))
    xtr = xt.rearrange("(c p) t -> p c t", p=P)
    # tile 0's x^T slice first (it gates the first q_proj), then W half 0
    # (q_proj for heads 0:8 only needs W cols 0:512), then the rest
    xt0_sb = xtp.tile([P, 8, P], FP16, tag="xt")
    # W as lhsT chunks, one tile per half so q_proj h0 only waits on its own
    # half's DMA: w_sb[h][p, c, j] = W[c*128 + p, h*512 + j]
    wqr = wq.rearrange("(c p) j -> p c j", p=P)
    w_sb = [const_pool.tile([P, 8, DH], FP16, name=f"w{h}") for h in range(2)]
    kp = ctx.enter_context(tc.tile_pool(name="k", bufs=2))
    # prologue DMAs all on the scalar queue in dependence order: the DMA
    # engines drain one queue FIFO, so this ordering controls arrival
    k0_ch = [kp.tile([P, 4 if lc == 0 else 8, D], FP16, tag=f"k{lc}",
                     name=f"k0{lc}") for lc in range(2)]
    bq_sb = const_pool.tile([1, D], FP16)
    ones_sb = const_pool.tile([1, P], FP16)
    with tc.high_priority(offset=200):
        nc.scalar.dma_start(bq_sb[:], bq.unsqueeze(0))
        nc.scalar.dma_start(ones_sb[:], ones.unsqueeze(0))
        nc.scalar.dma_start(xt0_sb[:], xtr[:, :, 0:P])
        nc.scalar.dma_start(w_sb[0][:], wqr[:, :, 0:DH])
        nc.scalar.dma_start(k0_ch[0][:], kh[0:P, 0:4, :])
        nc.scalar.dma_start(w_sb[1][:], wqr[:, :, DH:D])
        nc.scalar.dma_start(k0_ch[1][:], kh[0:P, 4:12, :])
    ident = const_pool.tile([P, P], FP16)
    masks.make_identity(nc, ident[:])
    qp = ctx.enter_context(tc.tile_pool(name="q", bufs=2))
    prodp = ctx.enter_context(tc.tile_pool(name="prod", bufs=3))
    p2p = ctx.enter_context(tc.tile_pool(name="p2", bufs=1))
    op = ctx.enter_context(tc.tile_pool(name="o", bufs=1))
    sp = ctx.enter_context(tc.tile_pool(name="smx", bufs=4))
    ps_q = ctx.enter_context(tc.tile_pool(name="ps_q", bufs=2, space="PSUM"))
    ps_a = ctx.enter_context(tc.tile_pool(name="ps_a", bufs=2, space="PSUM"))

    # PE warm-up: ~4us of dummy matmuls at t~0 so the HAM clock-gate opens
    # before tile 0's q_proj (cold PE is the prologue critical path)
    warm_ps = ps_q.tile([P, DH], FP32, tag="q0")
    for i in range(64):
        nc.tensor.matmul(
            warm_ps[:, 0:P], lhsT=ident[:], rhs=ident[:],
            start=(i == 0), stop=(i == 63),
        )

    def flush_pending(pending):
        # one-tile-deferred output drain: by now the PE sum-over-l matmuls
        # for that tile are long done, so ACT never stalls on the PE counter
        a_prev, tok_prev = pending
        o_sb = op.tile([P, D], FP32, tag="o")
        nc.scalar.copy(o_sb[:], a_prev[:])
        nc.sync.dma_start(out[tok_prev], o_sb[:])

    pending = None
    for it in range(NT * repeat):
        tt = it % NT
        tok = slice(tt * P, (tt + 1) * P)

        # ---- loads ----
        if it == 0:
            xt_sb = xt0_sb
        else:
            xt_sb = xtp.tile([P, 8, P], FP16, tag="xt")
            nc.scalar.dma_start(xt_sb[:], xtr[:, :, tok])
        # k in 3 chunk tiles of 4 layers each: dependency tracking is
        # per-tile, so the first scores-mul only waits on its own chunk
        if it == 0:
            k_ch = k0_ch
        else:
            k_ch = [kp.tile([P, 4 if lc == 0 else 8, D], FP16, tag=f"k{lc}",
                            name=f"k{lc}") for lc in range(2)]
            nc.sync.dma_start(k_ch[0][:], kh[tok, 0:4, :])
            nc.sync.dma_start(k_ch[1][:], kh[tok, 4:12, :])

        acc = ps_a.tile([P, D], FP32, tag="acc")

        for hh in range(2):
            hcol = slice(hh * DH, (hh + 1) * DH)      # feature columns
            q_ps = ps_q.tile([P, DH], FP32, tag=f"q{hh}")
            # q_proj for this half's output columns (token-major PSUM)
            with tc.high_priority(offset=180):
                for c in range(8):
                    nc.tensor.matmul(
                        q_ps[:],
                        lhsT=xt_sb[:, c, :],
                        rhs=w_sb[hh][:, c, :],
                        start=(c == 0),
                        stop=False,
                    )
                nc.tensor.matmul(
                    q_ps[:],
                    lhsT=ones_sb[:],
                    rhs=bq_sb[:, hcol],
                    start=False,
                    stop=True,
                )
                # q -> SBUF fp16 with 1/sqrt(hd) folded in, on ACT
                q16 = qp.tile([P, DH], FP16, tag=f"q{hh}")
                nc.scalar.mul(q16[:], q_ps[:], SCALE)

            if pending is not None:
                flush_pending(pending)
                pending = None

            # ---- scores: prod = k * q (broadcast over l), fold over hd ----
            qhe = q16[:].rearrange("p (h e) -> p h e", h=HH).unsqueeze(1)
            qv4 = qhe.broadcast_to([P, 4, HH, HD])
            qv8 = qhe.broadcast_to([P, 8, HH, HD])
            prod = prodp.tile([P, L, HH, HD], FP16, tag=f"prod{hh}")
            scr = sp.tile([P, L, HH], FP32, tag=f"scr{hh}")

            def fold(eng, ls):
                # in-place fold tree over hd: 64->32->...->2, then fp32 tail.
                # dst aliases in1 exactly (same element positions) which is
                # safe for the streaming engines.
                off = 0
                for w0 in (32, 16, 8, 4, 2):
                    eng.tensor_add(
                        prod[:, ls, :, off + w0:off + 2 * w0],
                        prod[:, ls, :, off:off + w0],
                        prod[:, ls, :, off + w0:off + 2 * w0],
                    )
                    off += w0
                eng.tensor_add(
                    scr[:, ls].unsqueeze(3),
                    prod[:, ls, :, 62:63],
                    prod[:, ls, :, 63:64],
                )

            # last unit runs all-DVE so the kernel tail isn't gated on the
            # slower gpsimd ops
            last_unit = (it == NT * repeat - 1)
            gf = 0 if last_unit else GF
            gp2 = 0 if last_unit else GP2

            with tc.high_priority(offset=60):
                for lc in range(3):
                    k4c = k_ch[lc][:, :, hcol].rearrange(
                        "p l (h e) -> p l h e", h=HH)
                    nc.vector.tensor_mul(
                        prod[:, 4 * lc:4 * lc + 4], k4c, qv4)
                if gf > 0:
                    fold(nc.gpsimd, slice(0, gf))
                if gf < L:
                    fold(nc.vector, slice(gf, L))

            # ---- softmax over l (no max subtraction) ----
            es = sp.tile([P, L, HH], FP32, tag=f"es{hh}")
            nc.scalar.activation(es[:], scr[:], mybir.ActivationFunctionType.Exp)
            den = sp.tile([P, HH], FP32, tag=f"den{hh}")
            nc.vector.tensor_reduce(
                den[:],
                es[:].rearrange("p l h -> p h l"),
                axis=mybir.AxisListType.X,
                op=mybir.AluOpType.add,
            )
            rd = sp.tile([P, HH], FP32, tag=f"rd{hh}")
            nc.vector.reciprocal(rd[:], den[:])

            # normalized weights into slots 0,1 of the expanded tile (an
            # fp16 pair = one fp32 word), then fp32-word broadcast on ACT.
            # wb aliases prod (consumed by the fold); prod bufs=3 keeps the
            # resulting WAR two units away from the next mul
            wb = prod
            rdv = rd[:].unsqueeze(1).broadcast_to([P, L, HH]).unsqueeze(3)
            nc.vector.tensor_mul(
                wb[:, :, :, 0:2],
                es[:].unsqueeze(3).broadcast_to([P, L, HH, 2]),
                rdv.broadcast_to([P, L, HH, 2]),
            )
            wbf = wb[:].bitcast(FP32)  # [P, L, HH, 32] fp32 words
            prod2 = p2p.tile([P, L, DH], FP16, tag=f"p2{hh}")
            wbflat = wb[:].rearrange("p l h e -> p l (h e)")

            def wsum_mm(l, first, last):
                nc.tensor.matmul(
                    acc[:, hcol],
                    lhsT=ident[:],
                    rhs=prod2[:, l, :],
                    start=first,
                    stop=last,
                )

            def kk(l0, l1):
                # layers [l0, l1) must live in one k chunk tile (split 4+8)
                if l1 <= 4:
                    return k_ch[0][:, l0:l1, hcol]
                assert l0 >= 4
                return k_ch[1][:, l0 - 4:l1 - 4, hcol]

            # gpsimd takes layers [0, gp2) in 4-layer chunk ops; DVE the
            # rest in 2-layer groups pipelined with the expansion copies.
            for lh in range(6):
                ls = slice(lh * 2, (lh + 1) * 2)
                nc.scalar.copy(
                    wbf[:, ls, :, 1:32],
                    wbf[:, ls, :, 0:1].broadcast_to([P, 2, HH, 31]),
                )
                if ls.start < gp2:   # gp takes this 2-layer group
                    g1 = min(ls.stop, gp2)
                    nc.gpsimd.tensor_mul(
                        prod2[:, ls.start:g1, :], kk(ls.start, g1),
                        wbflat[:, ls.start:g1, :]
                    )
                d0 = max(ls.start, gp2)
                while d0 < ls.stop:
                    d1 = min(ls.stop, 4 if d0 < 4 else 12)
                    nc.vector.tensor_mul(
                        prod2[:, d0:d1, :], kk(d0, d1), wbflat[:, d0:d1, :]
                    )
                    d0 = d1
                for l in range(max(ls.start, gp2), ls.stop):
                    wsum_mm(l, first=(l == gp2), last=(l == L - 1 and gp2 == 0))
            for l in range(gp2):
                wsum_mm(l, first=(gp2 == L), last=(l == gp2 - 1))

        pending = (acc, tok)

    flush_pending(pending)


_NC_CACHE = {}


def build_nc(repeat=1):
    if repeat in _NC_CACHE:
        return _NC_CACHE[repeat]
    nc = bacc.Bacc("TRN2", target_bir_lowering=False, debug=False,
                   num_devices=N_CORES)
    xt = nc.dram_tensor("xt", [D, T], FP16, kind="ExternalInput").ap()
    kh = nc.dram_tensor("kh", [T, L, D], FP16, kind="ExternalInput").ap()
    wq = nc.dram_tensor("wq", [D, D], FP16, kind="ExternalInput").ap()
    bq = nc.dram_tensor("bq", [D], FP16, kind="ExternalInput").ap()
    ones = nc.dram_tensor("ones", [P], FP16, kind="ExternalInput").ap()
    out = nc.dram_tensor("out", [T, D], FP32, kind="ExternalOutput").ap()
    with tile.TileContext(nc) as tc, ExitStack() as ctx:
        build_body(ctx, tc, out, xt, kh, wq, bq, ones, repeat=repeat)
    nc.compile()
    _NC_CACHE[repeat] = nc
    return nc


def make_in_maps(x_current, layer_history, W_q, b_q):
    x16 = np.asarray(x_current, dtype=np.float16).reshape(B * S, D)
    k16 = np.asarray(layer_history, dtype=np.float16).reshape(B * S, L, D)
    W16 = np.asarray(W_q, dtype=np.float16)
    b16 = np.asarray(b_q, dtype=np.float16)
    in_maps = []
    for c in range(N_CORES):
        sl = slice(c * T, (c + 1) * T)
        in_maps.append({
            "xt": np.ascontiguousarray(x16[sl].T),
            "kh": np.ascontiguousarray(k16[sl]),
            "wq": W16,
            "bq": b16,
            "ones": np.ones((P,), np.float16),
        })
    return in_maps


def kernel(x_current, layer_history, W_q, b_q):
    nc = build_nc()
    in_maps = make_in_maps(x_current, layer_history, W_q, b_q)
    res = run_bass_kernel_spmd(nc, in_maps, core_ids=list(range(N_CORES)))
    out = np.concatenate([res.results[c]["out"] for c in range(N_CORES)], axis=0)
    return out.reshape(B, S, D).astype(np.float32)


if __name__ == "__main__":
    rng = np.random.default_rng(0)
    x = rng.standard_normal((B, S, D), dtype=np.float32)
    k = rng.standard_normal((B, S, L, D), dtype=np.float32)
    W = (rng.standard_normal((D, D), dtype=np.float32) / math.sqrt(D)).astype(np.float32)
    b = (rng.standard_normal((D,), dtype=np.float32) * 0.01).astype(np.float32)
    o = kernel(x, k, W, b)
    print("ok", o.shape, o.dtype, float(np.abs(o).mean()))
